# revision 11
# baseline (speedup 1.0000x reference)
# Trainium2 Bass kernel for nn_MetaRNNBase (2-layer HyperLSTM, B=32 T=256 H=1024 HY=256 E=64)
# Strategy: tensor-parallel over 8 cores (gate/feature dim), f32 storage + float32r matmuls,
# layer-interleaved recurrent loop with per-step AllGather of h / hyper-h slices.
import sys, os, time
sys.path.insert(0, '/opt/trn_rl_repo')
import numpy as np

B, T_FULL, H, HY, E, L = 32, 256, 1024, 256, 64, 2
P = 128          # partitions
NC = 8           # cores
R = 8            # h-ring depth (rounds)
RH = 4           # hh-ring depth
D = 5            # L2 lag (rounds)
SLOT2GATE = [0, 1, 3, 2]   # our col order [i,f,o,g] -> orig gate index in (i,f,g,o)


# ---------------------------------------------------------------- host weight prep
def _gate_rows(n_per_gate, base_sel):
    idx = []
    for s in range(4):
        g = SLOT2GATE[s]
        idx.extend(g * n_per_gate + j for j in base_sel)
    return np.array(idx)


def prep_inputs(inputs, T):
    f32 = np.float32
    inp = {k: np.asarray(v, dtype=f32) for k, v in inputs.items()}
    x = inp['input'][:, :T]                       # (B,T,H)
    xT = np.ascontiguousarray(x.transpose(2, 1, 0)).reshape(8, 128, T * B)
    in_maps = []
    for r in range(NC):
        hr = np.arange(128 * r, 128 * (r + 1))
        g = r % 4
        yg = np.arange(64 * g, 64 * (g + 1))
        hsel = _gate_rows(H, hr)                  # 512 rows of 4H
        ysel = _gate_rows(HY, yg)                 # 256 rows of 4HY
        wih = np.stack([inp['w_ih'][l][hsel].T.reshape(8, 128, 512).transpose(1, 0, 2)
                        for l in range(L)])       # (L,128,8,512)
        wmain = np.stack([inp['w_hh'][l][hsel].T.reshape(8, 128, 512).transpose(1, 0, 2)
                          for l in range(L)])
        whih_h = np.stack([inp['hyper_w_ih'][l][ysel, H:].T.reshape(8, 128, 256).transpose(1, 0, 2)
                           for l in range(L)])    # (L,128,8,256)
        whih_x = np.stack([inp['hyper_w_ih'][l][ysel, :H].T.reshape(8, 128, 256).transpose(1, 0, 2)
                           for l in range(L)])
        whhh = np.stack([inp['hyper_w_hh'][l][ysel].T.reshape(2, 128, 256).transpose(1, 0, 2)
                         for l in range(L)])      # (L,128,2,256)
        wzd = np.zeros((L, 256, 1536), f32)
        bias_zd = np.zeros((L, 1, 1536), f32)
        for l in range(L):
            for ti, (wd_k, wz_k, bz_k) in enumerate(
                    [('w_dh', 'w_zh', 'b_zh'), ('w_dx', 'w_zx', 'b_zx'), ('w_db', 'w_zb', None)]):
                for s in range(4):
                    gg = SLOT2GATE[s]
                    wd = inp[wd_k][l, gg][hr]               # (128,E)
                    wz = inp[wz_k][l][gg * E:(gg + 1) * E]  # (E,HY)
                    M = (wd.astype(np.float64) @ wz.astype(np.float64)).T  # (HY,128)
                    wzd[l, :, ti * 512 + s * 128: ti * 512 + (s + 1) * 128] = M.astype(f32)
                    if bz_k is not None:
                        bz = inp[bz_k][l, gg * E:(gg + 1) * E]
                        bias_zd[l, 0, ti * 512 + s * 128: ti * 512 + (s + 1) * 128] = wd @ bz
            bias_zd[l, 0, 1024:1536] += inp['b0'][l][hsel]
        wzd = wzd.reshape(L, 2, 128, 1536).transpose(0, 2, 1, 3)   # (L,128,2,1536)
        bias_hyp = inp['hyper_b'][:, ysel].reshape(L, 1, 256)
        in_maps.append({
            'xT': xT,
            'wih': np.ascontiguousarray(wih),
            'wmain': np.ascontiguousarray(wmain),
            'whih_h': np.ascontiguousarray(whih_h),
            'whih_x': np.ascontiguousarray(whih_x),
            'whhh': np.ascontiguousarray(whhh),
            'wzd': np.ascontiguousarray(wzd),
            'bias_zd': np.ascontiguousarray(bias_zd),
            'bias_hyp': np.ascontiguousarray(bias_hyp),
        })
    return in_maps


# ---------------------------------------------------------------- numpy mirror (validation)
def numpy_forward(inputs, T):
    f32 = np.float32
    inp = {k: np.asarray(v, dtype=f32) for k, v in inputs.items()}
    x = inp['input'][:, :T]
    sig = lambda v: 1.0 / (1.0 + np.exp(-v))
    Ms, bs = [], []
    for l in range(L):
        Ml, bl = [], []
        for wd_k, wz_k, bz_k in [('w_dh', 'w_zh', 'b_zh'), ('w_dx', 'w_zx', 'b_zx'), ('w_db', 'w_zb', None)]:
            M = np.zeros((HY, 4 * H), f32)
            bias = np.zeros(4 * H, f32)
            for g in range(4):
                wd = inp[wd_k][l, g]
                wz = inp[wz_k][l][g * E:(g + 1) * E]
                M[:, g * H:(g + 1) * H] = (wd @ wz).T
                if bz_k is not None:
                    bias[g * H:(g + 1) * H] = wd @ inp[bz_k][l, g * E:(g + 1) * E]
            if bz_k is None:
                bias += inp['b0'][l]
            Ml.append(M); bl.append(bias)
        Ms.append(Ml); bs.append(bl)
    h = np.zeros((L, B, H), f32); c = np.zeros((L, B, H), f32)
    hh = np.zeros((L, B, HY), f32); hc = np.zeros((L, B, HY), f32)
    ys = []
    for t in range(T):
        xin = x[:, t]
        for l in range(L):
            hg = (xin @ inp['hyper_w_ih'][l, :, :H].T + h[l] @ inp['hyper_w_ih'][l, :, H:].T
                  + hh[l] @ inp['hyper_w_hh'][l].T + inp['hyper_b'][l])
            hi, hf, hgc, ho = np.split(hg, 4, 1)
            hc[l] = sig(hf) * hc[l] + sig(hi) * np.tanh(hgc)
            hh[l] = sig(ho) * np.tanh(hc[l])
            dh = hh[l] @ Ms[l][0] + bs[l][0]
            dx = hh[l] @ Ms[l][1] + bs[l][1]
            db = hh[l] @ Ms[l][2] + bs[l][2]
            pre = dh * (h[l] @ inp['w_hh'][l].T) + dx * (xin @ inp['w_ih'][l].T) + db
            i, f, gg, o = np.split(pre, 4, 1)
            c[l] = sig(f) * c[l] + sig(i) * np.tanh(gg)
            h[l] = sig(o) * np.tanh(c[l])
            xin = h[l]
        ys.append(h[L - 1].copy())
    return np.stack(ys, 1), h[L - 1], c[L - 1]


# ---------------------------------------------------------------- bass program
def build_program(T):
    import concourse.bass as bass
    import concourse.mybir as mybir
    import concourse.tile as tile
    from concourse import bacc
    from concourse.masks import make_identity
    DT = mybir.dt
    F32, F32R = DT.float32, DT.float32r
    AF = mybir.ActivationFunctionType

    nc = bacc.Bacc("TRN2", target_bir_lowering=False, debug=False,
                   enable_asserts=False, num_devices=NC)
    xT_d = nc.dram_tensor("xT", [8, 128, T * B], F32, kind="ExternalInput")
    wih_d = nc.dram_tensor("wih", [L, 128, 8, 512], F32, kind="ExternalInput")
    wmain_d = nc.dram_tensor("wmain", [L, 128, 8, 512], F32, kind="ExternalInput")
    whih_h_d = nc.dram_tensor("whih_h", [L, 128, 8, 256], F32, kind="ExternalInput")
    whih_x_d = nc.dram_tensor("whih_x", [L, 128, 8, 256], F32, kind="ExternalInput")
    whhh_d = nc.dram_tensor("whhh", [L, 128, 2, 256], F32, kind="ExternalInput")
    wzd_d = nc.dram_tensor("wzd", [L, 128, 2, 1536], F32, kind="ExternalInput")
    bias_zd_d = nc.dram_tensor("bias_zd", [L, 1, 1536], F32, kind="ExternalInput")
    bias_hyp_d = nc.dram_tensor("bias_hyp", [L, 1, 256], F32, kind="ExternalInput")
    out_h_d = nc.dram_tensor("out_h", [T, B, 128], F32, kind="ExternalOutput")
    out_c_d = nc.dram_tensor("out_c", [B, 128], F32, kind="ExternalOutput")
    xp1_d = nc.dram_tensor("xp1", [T, B, 512], F32)
    hxp1_d = nc.dram_tensor("hxp1", [T, B, 256], F32)

    with tile.TileContext(nc) as tc:
        with tc.tile_pool(name="wpool", bufs=1) as wp, \
             tc.tile_pool(name="work", bufs=2) as sp, \
             tc.tile_pool(name="psA", bufs=1, space="PSUM") as ppA, \
             tc.tile_pool(name="psB", bufs=2, space="PSUM") as ppB, \
             tc.tile_pool(name="dram", bufs=4, space="DRAM") as dp:

            def wload(name, dram_ap, shape):
                t = wp.tile(shape, F32R, tag=name, name=name)
                nc.gpsimd.dma_start(t[:], dram_ap)
                return t
            wmain_t = [wload(f"wmain{l}", wmain_d[l], [128, 8 * 512]) for l in range(L)]
            whih_h_t = [wload(f"whih_h{l}", whih_h_d[l], [128, 8 * 256]) for l in range(L)]
            whhh_t = [wload(f"whhh{l}", whhh_d[l], [128, 2 * 256]) for l in range(L)]
            wzd_t = [wload(f"wzd{l}", wzd_d[l], [128, 2 * 1536]) for l in range(L)]
            bias_zd_t = [wload(f"bias_zd{l}", bias_zd_d[l], [1, 1536]) for l in range(L)]
            bias_hyp_t = [wload(f"bias_hyp{l}", bias_hyp_d[l], [1, 256]) for l in range(L)]
            def wload_tag(tag, name, dram_ap, shape):
                t = wp.tile(shape, F32R, tag=tag, name=name)
                nc.gpsimd.dma_start(t[:], dram_ap)
                return t
            wih1_t = wload_tag("wih_sh", "wih1", wih_d[0], [128, 8 * 512])
            whih_x1_t = wload_tag("whx_sh", "whih_x1", whih_x_d[0], [128, 8 * 256])

            ones_t = wp.tile([1, 128], F32R, tag="ones", name="ones")
            nc.vector.memset(ones_t[:].bitcast(F32), 1.0)
            ident = wp.tile([32, 32], F32, tag="ident", name="ident")
            make_identity(nc, ident[:])
            zeros_t = wp.tile([128, 64], F32, tag="zeros", name="zeros")
            nc.vector.memset(zeros_t[:], 0.0)

            ring_h = [wp.tile([128, R * 256], F32R, tag=f"ring_h{l}", name=f"ring_h{l}") for l in range(L)]
            ring_hh = [wp.tile([128, RH * 64], F32R, tag=f"ring_hh{l}", name=f"ring_hh{l}") for l in range(L)]
            for l in range(L):
                nc.vector.memset(ring_h[l][:, (R - 1) * 256:].bitcast(F32), 0.0)
                nc.vector.memset(ring_hh[l][:, (RH - 1) * 64:].bitcast(F32), 0.0)
            c_st = [None, None]
            hc_st = [None, None]
            for l in range(L):
                c_st[l] = sp.tile([32, 128], F32, tag=f"c{l}", name=f"c{l}")
                hc_st[l] = sp.tile([32, 64], F32, tag=f"hc{l}", name=f"hc{l}")
                nc.vector.memset(c_st[l][:], 0.0)
                nc.vector.memset(hc_st[l][:], 0.0)

            # ---- precompute L1 x-projections
            for m in range(T // 4):
                lx = sp.tile([128, 8 * 128], F32R, tag="lx", name="lx")
                nc.gpsimd.dma_start(lx[:], xT_d[:, :, m * 128:(m + 1) * 128]
                                    .rearrange("c p m -> p c m"))
                ps = ppA.tile([128, 768], F32, tag="xpb", name="xpb")
                for c in range(8):
                    nc.tensor.matmul(ps[:, 0:512], lx[:, c * 128:(c + 1) * 128],
                                     wih1_t[:, c * 512:(c + 1) * 512],
                                     start=(c == 0), stop=(c == 7))
                for c in range(8):
                    nc.tensor.matmul(ps[:, 512:768], lx[:, c * 128:(c + 1) * 128],
                                     whih_x1_t[:, c * 256:(c + 1) * 256],
                                     start=(c == 0), stop=False)
                nc.tensor.matmul(ps[:, 512:768], ones_t[:, 0:128], bias_hyp_t[0][:],
                                 start=False, stop=True)
                cps = sp.tile([128, 768], F32, tag="cps", name="cps", bufs=1)
                nc.scalar.copy(cps[:, 0:512], ps[:, 0:512])
                nc.scalar.copy(cps[:, 512:768], ps[:, 512:768])
                nc.sync.dma_start(xp1_d[4 * m:4 * m + 4].rearrange("t b n -> (t b) n"),
                                  cps[:, 0:512])
                nc.sync.dma_start(hxp1_d[4 * m:4 * m + 4].rearrange("t b n -> (t b) n"),
                                  cps[:, 512:768])

            wih2_t = wload_tag("wih_sh", "wih2", wih_d[1], [128, 8 * 512])
            whih_x2_t = wload_tag("whx_sh", "whih_x2", whih_x_d[1], [128, 8 * 256])

            # ---- recurrent loop
            xp2_batches = {}

            def cell_head(l, q1, qh1, hxp_ap, tr_tile):
                hyp = ppA.tile([32, 256], F32, tag="hyp", name="hyp")
                for cc in range(8):
                    nc.tensor.matmul(hyp[:], ring_h[l][:, q1 * 256 + cc * 32: q1 * 256 + (cc + 1) * 32],
                                     whih_h_t[l][:, cc * 256:(cc + 1) * 256],
                                     start=(cc == 0), stop=False)
                for cc in range(2):
                    nc.tensor.matmul(hyp[:], ring_hh[l][:, qh1 * 64 + cc * 32: qh1 * 64 + (cc + 1) * 32],
                                     whhh_t[l][:, cc * 256:(cc + 1) * 256],
                                     start=False, stop=(cc == 1))
                hmm = ppA.tile([32, 512], F32, tag="hmm", name="hmm")
                for cc in range(8):
                    nc.tensor.matmul(hmm[:], ring_h[l][:, q1 * 256 + cc * 32: q1 * 256 + (cc + 1) * 32],
                                     wmain_t[l][:, cc * 512:(cc + 1) * 512],
                                     start=(cc == 0), stop=(cc == 7))
                hmm_s = sp.tile([32, 512], F32, tag="hmm_s", name="hmm_s")
                nc.scalar.copy(hmm_s[:], hmm[:])
                u = sp.tile([32, 256], F32, tag="u", name="u")
                nc.vector.tensor_add(u[:], hyp[:], hxp_ap)
                s = sp.tile([32, 256], F32, tag="s", name="s")
                nc.scalar.activation(s[:, 0:192], u[:, 0:192], AF.Sigmoid)
                nc.scalar.activation(s[:, 192:256], u[:, 192:256], AF.Tanh)
                t1 = sp.tile([32, 64], F32, tag="t1", name="t1")
                t2 = sp.tile([32, 64], F32, tag="t2", name="t2")
                nc.gpsimd.tensor_mul(t1[:], s[:, 64:128], hc_st[l][:])
                nc.gpsimd.tensor_mul(t2[:], s[:, 0:64], s[:, 192:256])
                hc2 = sp.tile([32, 64], F32, tag=f"hc{l}", name=f"hc{l}")
                nc.gpsimd.tensor_add(hc2[:], t1[:], t2[:])
                hc_st[l] = hc2
                th = sp.tile([32, 64], F32, tag="th", name="th")
                nc.scalar.activation(th[:], hc2[:], AF.Tanh)
                hh2 = sp.tile([32, 64], F32, tag="hh2", name="hh2")
                nc.gpsimd.tensor_mul(hh2[:], s[:, 128:192], th[:])
                nc.tensor.transpose(tr_tile[0:64, 0:32], hh2[:], ident[:])
                return hmm_s

            def cell_tail(l, qz, hmm_s, xp_ap, tr_tile):
                psd = []
                for ti in range(3):
                    z = ppB.tile([32, 512], F32, tag="zd", name="zd")
                    for cc in range(2):
                        nc.tensor.matmul(z[:], ring_hh[l][:, qz * 64 + cc * 32: qz * 64 + (cc + 1) * 32],
                                         wzd_t[l][:, cc * 1536 + ti * 512: cc * 1536 + (ti + 1) * 512],
                                         start=(cc == 0), stop=False)
                    nc.tensor.matmul(z[:], ones_t[0:1, 0:32], bias_zd_t[l][0:1, ti * 512:(ti + 1) * 512],
                                     start=False, stop=True)
                    psd.append(z)
                dh, dx, db = psd
                p1 = sp.tile([32, 512], F32, tag="p1", name="p1", bufs=1)
                nc.vector.tensor_mul(p1[:], dh[:], hmm_s[:])
                p2 = sp.tile([32, 512], F32, tag="p2", name="p2", bufs=1)
                nc.vector.tensor_mul(p2[:], dx[:], xp_ap)
                p3 = sp.tile([32, 512], F32, tag="p3", name="p3", bufs=1)
                nc.vector.tensor_add(p3[:], p1[:], p2[:])
                p4 = sp.tile([32, 512], F32, tag="p4", name="p4", bufs=1)
                nc.vector.tensor_add(p4[:], p3[:], db[:])
                sg = sp.tile([32, 512], F32, tag="sg", name="sg", bufs=1)
                nc.scalar.activation(sg[:, 0:384], p4[:, 0:384], AF.Sigmoid)
                nc.scalar.activation(sg[:, 384:512], p4[:, 384:512], AF.Tanh)
                a1 = sp.tile([32, 128], F32, tag="a1", name="a1")
                a2 = sp.tile([32, 128], F32, tag="a2", name="a2")
                nc.gpsimd.tensor_mul(a1[:], sg[:, 128:256], c_st[l][:])
                nc.gpsimd.tensor_mul(a2[:], sg[:, 0:128], sg[:, 384:512])
                c2 = sp.tile([32, 128], F32, tag=f"c{l}", name=f"c{l}")
                nc.gpsimd.tensor_add(c2[:], a1[:], a2[:])
                c_st[l] = c2
                tc2 = sp.tile([32, 128], F32, tag="tc2", name="tc2")
                nc.scalar.activation(tc2[:], c2[:], AF.Tanh)
                h2 = sp.tile([32, 128], F32, tag=f"h2_{l}", name=f"h2_{l}")
                nc.gpsimd.tensor_mul(h2[:], sg[:, 256:384], tc2[:])
                nc.tensor.transpose(tr_tile[:, 32:64], h2[:], ident[:])
                return h2, c2

            for k in range(T + D):
                l1_active = k < T
                l2_active = 0 <= k - D < T
                j = k - D
                q1 = (k - 1) % R
                qh1 = (k - 1) % RH
                qz = k % RH

                if k >= 4 and k % 4 == 0 and (k - 4) < T:
                    m0 = k - 4
                    s0 = m0 % R
                    lx2 = sp.tile([128, 8 * 128], F32R, tag="lx2", name="lx2", bufs=1)
                    src = ring_h[0][:].rearrange("p (q c b) -> p q c b", q=R, c=8)
                    for cc in range(8):
                        nc.vector.tensor_copy(
                            lx2[:, cc * 128:(cc + 1) * 128].rearrange("p (q b) -> p q b", q=4),
                            src[:, s0:s0 + 4, cc, :])
                    ps2 = ppA.tile([128, 768], F32, tag="xpb", name="xpb")
                    for cc in range(8):
                        nc.tensor.matmul(ps2[:, 0:512], lx2[:, cc * 128:(cc + 1) * 128],
                                         wih2_t[:, cc * 512:(cc + 1) * 512],
                                         start=(cc == 0), stop=(cc == 7))
                    for cc in range(8):
                        nc.tensor.matmul(ps2[:, 512:768], lx2[:, cc * 128:(cc + 1) * 128],
                                         whih_x2_t[:, cc * 256:(cc + 1) * 256],
                                         start=(cc == 0), stop=False)
                    nc.tensor.matmul(ps2[:, 512:768], ones_t[:, 0:128], bias_hyp_t[1][:],
                                     start=False, stop=True)
                    xp2 = sp.tile([128, 512], F32, tag="xp2", name="xp2")
                    nc.scalar.copy(xp2[:], ps2[:, 0:512])
                    hxp2 = sp.tile([128, 256], F32, tag="hxp2", name="hxp2")
                    nc.scalar.copy(hxp2[:], ps2[:, 512:768])
                    xp2_batches[m0 // 4] = (xp2, hxp2)
                    if m0 // 4 >= 2:
                        xp2_batches.pop(m0 // 4 - 2, None)

                tr1 = ppB.tile([128, 64], F32, tag="tr", name="tr")
                tr2 = ppB.tile([128, 64], F32, tag="tr", name="tr")
                hmm1 = hmm2 = None
                xp1_t = None
                if l1_active:
                    xp1_t = sp.tile([32, 512], F32, tag="xp1", name="xp1", bufs=3)
                    nc.sync.dma_start(xp1_t[:], xp1_d[k])
                    hxp1_t = sp.tile([32, 256], F32, tag="hxp1", name="hxp1", bufs=3)
                    nc.sync.dma_start(hxp1_t[:], hxp1_d[k])
                    hmm1 = cell_head(0, q1, qh1, hxp1_t[:], tr1)
                if l2_active:
                    jj = j % 4
                    xp2, hxp2 = xp2_batches[j // 4]
                    hmm2 = cell_head(1, q1, qh1, hxp2[32 * jj:32 * (jj + 1), :], tr2)

                stg_hh = sp.tile([128, 32], F32, tag="stg_hh", name="stg_hh")
                if l1_active:
                    nc.scalar.copy(stg_hh[0:64], tr1[0:64, 0:32])
                else:
                    nc.vector.memset(stg_hh[0:64], 0.0)
                if l2_active:
                    nc.scalar.copy(stg_hh[64:128], tr2[0:64, 0:32])
                else:
                    nc.vector.memset(stg_hh[64:128], 0.0)
                inb_hh = dp.tile([128, 32], F32, tag="inb_hh", name="inb_hh")
                nc.sync.dma_start(inb_hh[:], stg_hh[:])
                outb_hh = dp.tile([512, 32], F32, tag="outb_hh", name="outb_hh")
                nc.gpsimd.collective_compute(
                    "AllGather", mybir.AluOpType.bypass,
                    replica_groups=[[0, 1, 2, 3], [4, 5, 6, 7]],
                    ins=[inb_hh.opt()], outs=[outb_hh.opt()])
                # ring_hh[l] slot qz chunk c <- outb rows [128*(2c)+64l:+64], [128*(2c+1)+64l:+64]
                obhh = outb_hh[:].rearrange("(g u p) b -> g u p b", g=4, u=2)
                for l in range(L):
                    for c in range(2):
                        nc.gpsimd.dma_start(
                            ring_hh[l][:, qz * 64 + c * 32: qz * 64 + (c + 1) * 32],
                            obhh[2 * c:2 * c + 2, l])

                if l1_active:
                    cell_tail(0, qz, hmm1, xp1_t[:], tr1)
                if l2_active:
                    jj = j % 4
                    xp2, _ = xp2_batches[j // 4]
                    h2o, c2o = cell_tail(1, qz, hmm2, xp2[32 * jj:32 * (jj + 1), :], tr2)
                    nc.sync.dma_start(out_h_d[j], h2o[:])
                    if j == T - 1:
                        nc.sync.dma_start(out_c_d[:], c2o[:])

                stg_h = sp.tile([128, 64], F32, tag="stg_h", name="stg_h")
                if l1_active:
                    nc.scalar.copy(stg_h[:, 0:32], tr1[:, 32:64])
                else:
                    nc.vector.memset(stg_h[:, 0:32], 0.0)
                if l2_active:
                    nc.scalar.copy(stg_h[:, 32:64], tr2[:, 32:64])
                else:
                    nc.vector.memset(stg_h[:, 32:64], 0.0)
                inb_h = dp.tile([128, 64], F32, tag="inb_h", name="inb_h")
                nc.sync.dma_start(inb_h[:], stg_h[:])
                outb_h = dp.tile([1024, 64], F32, tag="outb_h", name="outb_h")
                nc.gpsimd.collective_compute(
                    "AllGather", mybir.AluOpType.bypass,
                    replica_groups=[[0, 1, 2, 3, 4, 5, 6, 7]],
                    ins=[inb_h.opt()], outs=[outb_h.opt()])
                qn = k % R
                obh = outb_h[:].rearrange("(c p) b -> p c b", c=8)
                for l in range(L):
                    nc.gpsimd.dma_start(
                        ring_h[l][:, qn * 256:(qn + 1) * 256],
                        obh[:, :, 32 * l:32 * (l + 1)])
    nc.compile()
    return nc


# ---------------------------------------------------------------- public entry
def run_on_hw(inputs, T, trace=False):
    from concourse.bass_utils import run_bass_kernel_spmd
    nc = build_program(T)
    in_maps = prep_inputs(inputs, T)
    res = run_bass_kernel_spmd(nc, in_maps, list(range(NC)), trace=trace)
    h_seq = np.zeros((B, T, H), np.float32)
    c_fin = np.zeros((B, H), np.float32)
    for r in range(NC):
        oh = res.results[r]['out_h']
        h_seq[:, :, 128 * r:128 * (r + 1)] = oh.transpose(1, 0, 2)
        c_fin[:, 128 * r:128 * (r + 1)] = res.results[r]['out_c']
    h_fin = h_seq[:, -1, :].copy()
    return (h_seq, h_fin, c_fin), res


def kernel(**inputs):
    out, _ = run_on_hw(inputs, inputs['input'].shape[1])
    return out
